# revision 19
# baseline (speedup 1.0000x reference)
"""Trainium2 Bass kernel v2: batched HMM log-forward via overlapped
forward-chain scan.

Linear-space chain over 64 states (state 0 bookend dropped):
  a_1 = E[:,obs_0]*trans0,  a_{t} = D_{E[:,obs_t]} Ttil^T a_{t-1},
  Z*e^{99+8192*C} = w~ . a_{8191}   (E drift-compensated by C_SHIFT).

Scheme: chains of length N_STEPS cover windows (a_j, a_j+N_STEPS],
a_j = j*STRIDE, plus a final chain ending exactly at 8191. Chain 0 starts
from the exact init; later chains from a warm init vector. After TAU
warmup steps a chain's state is parallel to the true alpha direction
(Perron-Frobenius contraction), so the scalar
  rho_j = <snap_j(tau_j), fin_{j-1}> / <fin_{j-1}, fin_{j-1}>
links consecutive chains and logZ telescopes: logZ = -sum_j log rho_j
+ log(w~ . fin_last) - 8192*C_SHIFT - 99.

Work vs fwd+bwd pairing drops ~1.45x (redundancy n/(n-tau) vs 2.0).

Device: 2 fwd chains per 128-partition column (block-diag weight
diag(Ttil,Ttil)), scan N_STEPS iterations over NCOL columns in G=2
groups; per step per group one PE matmul -> PSUM and one DVE multiply
PSUM*e(fp8) -> SBUF bf16. Snapshots at local steps TAU and TAU_F plus
the final state are DMA'd out; the stitch runs on host in float64.
"""

import os
import numpy as np
import ml_dtypes

B, T, S, V = 128, 8192, 65, 1024
N_CORES = 8
SEQ_PER_CORE = B // N_CORES  # 16
C_SHIFT = 6.9418
BF16 = ml_dtypes.bfloat16
FP8 = ml_dtypes.float8_e5m2

N_STEPS = int(os.environ.get("HMM_N", "65"))
TAU = int(os.environ.get("HMM_TAU", "1"))
STRIDE = N_STEPS - TAU
TM = T - 1  # matrices t = 1..8191


def _chain_starts():
    k_int = (TM - N_STEPS) // STRIDE
    starts = [j * STRIDE for j in range(k_int + 1)]
    if starts[-1] + N_STEPS < TM:
        starts.append(TM - N_STEPS)
    return starts


STARTS = _chain_starts()
NCH = len(STARTS)  # chains per sequence
TAU_F = STARTS[-2] + N_STEPS - STARTS[-1] if STARTS[-1] != STARTS[-2] + STRIDE else TAU
NCHAIN_CORE = NCH * SEQ_PER_CORE
assert NCHAIN_CORE % 2 == 0
NCOL = NCHAIN_CORE // 2  # 2 chains per column


def _strip_self_wait_events(nc):
    """Remove InstEventSemaphore instrs that only wait on the issuing
    engine's own semaphore (trivially-true WAW guards)."""
    eng_prefix = {
        "EngineType.DVE": "DVE_",
        "EngineType.PE": "PE_",
        "EngineType.Activation": "Activation_",
        "EngineType.Pool": "Pool_",
    }
    removed = 0
    for fn in nc.m.functions:
        for blk in fn.blocks:
            keep = []
            for inst in blk.instructions:
                if type(inst).__name__ == "InstEventSemaphore":
                    pfx = eng_prefix.get(str(getattr(inst, "engine", "")), None)
                    si = inst.sync_info
                    if (
                        pfx is not None
                        and si
                        and not si.on_update
                        and si.on_wait
                        and all(
                            w.ant_name.startswith(pfx)
                            and w.wait_mode == "sem-ge-imm"
                            for w in si.on_wait
                        )
                    ):
                        removed += 1
                        continue
                keep.append(inst)
            blk.instructions[:] = keep
    return removed


def _build_program(n_steps, n_groups=2):
    import contextlib
    import concourse.tile as tile
    from concourse import bacc, mybir

    nc = bacc.Bacc(None)
    gsz = [NCOL // n_groups + (1 if i < NCOL % n_groups else 0) for i in range(n_groups)]
    ranges = []
    lo = 0
    for g in gsz:
        ranges.append((lo, lo + g))
        lo += g

    w_dram = nc.declare_dram_parameter("wmat", [128, 128], mybir.dt.bfloat16, False)
    x0_dram = nc.declare_dram_parameter("x0", [128, NCOL], mybir.dt.bfloat16, False)
    e_dram = nc.declare_dram_parameter(
        "econg", [128, n_steps * NCOL], mybir.dt.float8e5, False
    )
    snap1_dram = nc.declare_dram_parameter("snap1", [128, NCOL], mybir.dt.bfloat16, True)
    snap2_dram = nc.declare_dram_parameter("snap2", [128, NCOL], mybir.dt.bfloat16, True)
    out_dram = nc.declare_dram_parameter("xout", [128, NCOL], mybir.dt.bfloat16, True)

    # e-stream chunk schedule: small first chunks so the scan starts early
    sched = []
    left = n_steps
    for sz in (1, 1, 2, 4, 8):
        if left > 0:
            sched.append(min(sz, left))
            left -= sched[-1]
    while left > 0:
        sched.append(min(16, left))
        left -= sched[-1]
    chunk_of_step = []
    for ci, sz in enumerate(sched):
        chunk_of_step += [ci] * sz
    chunk_base = np.cumsum([0] + sched[:-1])

    with tile.TileContext(nc) as tc:
        with contextlib.ExitStack() as ctx:
            const_pool = ctx.enter_context(tc.tile_pool(name="const", bufs=1))
            epool = ctx.enter_context(tc.tile_pool(name="emis", bufs=1))
            xpool = ctx.enter_context(tc.tile_pool(name="x", bufs=4))
            psum_pool = ctx.enter_context(
                tc.tile_pool(name="ps", bufs=2, space="PSUM")
            )
            fin_pool = ctx.enter_context(tc.tile_pool(name="fin", bufs=1))

            # first e-chunk at the head of the sync queue (it gates the
            # first DVE multiply; x0/w are quick and follow), remaining
            # chunks alternate sync/gpsimd queues
            e_tiles = []
            for ci, sz in enumerate(sched):
                et = epool.tile([128, sz * NCOL], mybir.dt.float8e5, tag=f"e{ci}")
                e_tiles.append(et)
            nc.sync.dma_start(e_tiles[0][:], e_dram[:, 0 : sched[0] * NCOL])
            x0_sb = const_pool.tile([128, NCOL], mybir.dt.bfloat16, tag="x0")
            nc.sync.dma_start(x0_sb[:], x0_dram[:])
            w_sb = const_pool.tile([128, 128], mybir.dt.bfloat16, tag="w")
            nc.sync.dma_start(w_sb[:], w_dram[:])
            for ci in range(1, len(sched)):
                lo = int(chunk_base[ci]) * NCOL
                eng = nc.sync if ci % 2 == 0 else nc.gpsimd
                eng.dma_start(
                    e_tiles[ci][:], e_dram[:, lo : lo + sched[ci] * NCOL]
                )

            dummy = fin_pool.tile([1, 4], mybir.dt.bfloat16, tag="dummy")

            xs = [(x0_sb, lo) for (lo, hi) in ranges]
            seen_chunk = -1
            for k in range(n_steps):
                step = k + 1  # local step index (1-based)
                ci = chunk_of_step[k]
                off = (k - int(chunk_base[ci])) * NCOL
                for g, (lo, hi) in enumerate(ranges):
                    cw = hi - lo
                    xt, xo = xs[g]
                    ps = psum_pool.tile([128, cw], mybir.dt.float32, tag=f"ps{g}")
                    for sub in range(0, cw, 512):
                        se = min(cw, sub + 512)
                        nc.tensor.matmul(
                            ps[:, sub:se],
                            w_sb[:],
                            xt[:, xo + sub : xo + se],
                            start=True,
                            stop=True,
                        )
                    xn = xpool.tile([128, cw], mybir.dt.bfloat16, tag=f"x{g}")
                    nc.vector.tensor_mul(
                        xn[:], ps[:], e_tiles[ci][:, off + lo : off + hi]
                    )
                    xs[g] = (xn, 0)
                if step == TAU:
                    for g, (lo, hi) in enumerate(ranges):
                        xt, xo = xs[g]
                        nc.sync.dma_start(
                            snap1_dram[:, lo:hi], xt[:, xo : xo + (hi - lo)]
                        )
                if step == TAU_F:
                    for g, (lo, hi) in enumerate(ranges):
                        xt, xo = xs[g]
                        nc.sync.dma_start(
                            snap2_dram[:, lo:hi], xt[:, xo : xo + (hi - lo)]
                        )

            for g, (lo, hi) in enumerate(ranges):
                xt, xo = xs[g]
                nc.sync.dma_start(out_dram[:, lo:hi], xt[:, xo : xo + (hi - lo)])

    nc.compile()
    if os.environ.get("HMM_STRIP_EV", "1") == "1":
        _strip_self_wait_events(nc)
    return nc


def _host_tables():
    """Chain->column mapping tables.

    Chain (seq b, j) for j in 0..NCH-1. Column layout: col = b*(NCH//2) + p
    holds chain 2p (partitions 0:64) and 2p+1 (partitions 64:128) of seq b
    when NCH even; handled generically via flat chain index."""
    # flat chain index c = b*NCH + j; column = c // 2, half = c % 2
    return None


def _host_prep(log_trans, log_emit, obvs):
    log_trans = np.asarray(log_trans, dtype=np.float64)
    log_emit = np.asarray(log_emit, dtype=np.float64)
    obvs = np.asarray(obvs).astype(np.int64)

    Ttil = np.exp(log_trans[1:, 1:])
    trans0 = np.exp(log_trans[0, 1:])
    E = np.exp(log_emit[1:, :] + C_SHIFT)  # [64,1024]
    E8 = E.astype(FP8)

    wmat = np.zeros((128, 128), dtype=np.float64)
    wmat[0:64, 0:64] = Ttil
    wmat[64:128, 64:128] = Ttil
    wmat = wmat.astype(BF16)

    # warm init: Perron vector of the mean-emission operator
    M = E.mean(axis=1)[:, None] * Ttil.T
    v = np.full(64, 1.0 / 64)
    for _ in range(100):
        v = M @ v
        v /= v.sum()
    warm = v

    starts = np.array(STARTS)  # [NCH]
    i_idx = np.arange(1, N_STEPS + 1)
    tok = starts[:, None] + i_idx[None, :]  # [NCH, n] global t per local step

    per_core = []
    for m in range(N_CORES):
        s0 = m * SEQ_PER_CORE
        obs_c = obvs[s0 : s0 + SEQ_PER_CORE, :]  # [16, T]

        # x0: [128, NCOL]; flat chain c = b*NCH + j -> column c//2, half c%2
        x0f = np.tile(warm[:, None], (1, SEQ_PER_CORE * NCH))
        x0f = x0f.reshape(64, SEQ_PER_CORE, NCH).copy()
        x0f[:, :, 0] = E[:, obs_c[:, 0]] * trans0[:, None]
        x0f = x0f.reshape(64, SEQ_PER_CORE * NCH)  # [64, nchains] b-major
        x0 = np.empty((128, NCOL), dtype=np.float64)
        x0[0:64, :] = x0f[:, 0::2]
        x0[64:128, :] = x0f[:, 1::2]
        per = {"wmat": wmat, "x0": x0.astype(BF16)}

        # e-stream: [128, n*NCOL] step-major fp8
        # chain c at local step i uses E8[:, obs[b, tok[j, i]]]
        em = E8[:, obs_c[:, tok]]  # [64, 16, NCH, n]
        em = em.reshape(64, SEQ_PER_CORE * NCH, N_STEPS)  # [64, nchains, n]
        econg = np.empty((128, N_STEPS, NCOL), dtype=FP8)
        econg[0:64] = np.transpose(em[:, 0::2, :], (0, 2, 1))
        econg[64:128] = np.transpose(em[:, 1::2, :], (0, 2, 1))
        per["econg"] = np.ascontiguousarray(econg.reshape(128, N_STEPS * NCOL))
        per_core.append(per)
    return per_core, warm


def _unpack_cols(arr):
    """[128, NCOL] -> [64, nchains] (b-major flat chain order)."""
    nch_flat = NCOL * 2
    out = np.empty((64, nch_flat), dtype=np.float64)
    out[:, 0::2] = arr[0:64, :]
    out[:, 1::2] = arr[64:128, :]
    return out


def _host_stitch(results, log_trans):
    log_trans = np.asarray(log_trans, dtype=np.float64)
    w_til = np.exp(log_trans[1:, 0] + 99.0)

    logZ = np.zeros(B, dtype=np.float64)
    for m in range(N_CORES):
        fin = _unpack_cols(np.asarray(results[m]["xout"], dtype=np.float64))
        sn1 = _unpack_cols(np.asarray(results[m]["snap1"], dtype=np.float64))
        sn2 = _unpack_cols(np.asarray(results[m]["snap2"], dtype=np.float64))
        fin = fin.reshape(64, SEQ_PER_CORE, NCH)
        sn1 = sn1.reshape(64, SEQ_PER_CORE, NCH)
        sn2 = sn2.reshape(64, SEQ_PER_CORE, NCH)
        for bb in range(SEQ_PER_CORE):
            logL = 0.0
            for j in range(1, NCH):
                snap = sn2[:, bb, j] if (j == NCH - 1 and TAU_F != TAU) else sn1[:, bb, j]
                fp = fin[:, bb, j - 1]
                rho = np.dot(snap, fp) / np.dot(fp, fp)
                logL -= np.log(rho)
            z = np.dot(w_til, fin[:, bb, NCH - 1])
            logZ[m * SEQ_PER_CORE + bb] = logL + np.log(z) - T * C_SHIFT - 99.0
    return logZ


def _run(nc, per_core, trace=False):
    from concourse.bass_utils import run_bass_kernel_spmd

    return run_bass_kernel_spmd(
        nc, per_core, list(range(N_CORES)), trace=trace, trace_cores=[0]
    )


def kernel(log_trans, log_emit, log_pi, obvs):
    nc = _build_program(N_STEPS)
    per_core, _ = _host_prep(log_trans, log_emit, obvs)
    res = _run(nc, per_core)
    out = _host_stitch(res.results, log_trans)
    return out.astype(np.float32)


# revision 21
# speedup vs baseline: 1.0390x; 1.0390x over previous
"""Trainium2 Bass kernel v2: batched HMM log-forward via overlapped
forward-chain scan.

Linear-space chain over 64 states (state 0 bookend dropped):
  a_1 = E[:,obs_0]*trans0,  a_{t} = D_{E[:,obs_t]} Ttil^T a_{t-1},
  Z*e^{99+8192*C} = w~ . a_{8191}   (E drift-compensated by C_SHIFT).

Scheme: chains of length N_STEPS cover windows (a_j, a_j+N_STEPS],
a_j = j*STRIDE, plus a final chain ending exactly at 8191. Chain 0 starts
from the exact init; later chains from a warm init vector. After TAU
warmup steps a chain's state is parallel to the true alpha direction
(Perron-Frobenius contraction), so the scalar
  rho_j = <snap_j(tau_j), fin_{j-1}> / <fin_{j-1}, fin_{j-1}>
links consecutive chains and logZ telescopes: logZ = -sum_j log rho_j
+ log(w~ . fin_last) - 8192*C_SHIFT - 99.

Work vs fwd+bwd pairing drops ~1.45x (redundancy n/(n-tau) vs 2.0).

Device: 2 fwd chains per 128-partition column (block-diag weight
diag(Ttil,Ttil)), scan N_STEPS iterations over NCOL columns in G=2
groups; per step per group one PE matmul -> PSUM and one DVE multiply
PSUM*e(fp8) -> SBUF bf16. Snapshots at local steps TAU and TAU_F plus
the final state are DMA'd out; the stitch runs on host in float64.
"""

import os
import numpy as np
import ml_dtypes

B, T, S, V = 128, 8192, 65, 1024
N_CORES = 8
SEQ_PER_CORE = B // N_CORES  # 16
C_SHIFT = 6.9418
BF16 = ml_dtypes.bfloat16
FP8 = ml_dtypes.float8_e5m2

N_STEPS = int(os.environ.get("HMM_N", "65"))
TAU = int(os.environ.get("HMM_TAU", "1"))
STRIDE = N_STEPS - TAU
TM = T - 1  # matrices t = 1..8191


def _chain_starts():
    k_int = (TM - N_STEPS) // STRIDE
    starts = [j * STRIDE for j in range(k_int + 1)]
    if starts[-1] + N_STEPS < TM:
        starts.append(TM - N_STEPS)
    return starts


STARTS = _chain_starts()
NCH = len(STARTS)  # chains per sequence
TAU_F = STARTS[-2] + N_STEPS - STARTS[-1] if STARTS[-1] != STARTS[-2] + STRIDE else TAU
NCHAIN_CORE = NCH * SEQ_PER_CORE
assert NCHAIN_CORE % 2 == 0
NCOL = NCHAIN_CORE // 2  # 2 chains per column


def _strip_self_wait_events(nc):
    """Remove InstEventSemaphore instrs that only wait on the issuing
    engine's own semaphore (trivially-true WAW guards)."""
    eng_prefix = {
        "EngineType.DVE": "DVE_",
        "EngineType.PE": "PE_",
        "EngineType.Activation": "Activation_",
        "EngineType.Pool": "Pool_",
    }
    removed = 0
    for fn in nc.m.functions:
        for blk in fn.blocks:
            keep = []
            for inst in blk.instructions:
                if type(inst).__name__ == "InstEventSemaphore":
                    pfx = eng_prefix.get(str(getattr(inst, "engine", "")), None)
                    si = inst.sync_info
                    if (
                        pfx is not None
                        and si
                        and not si.on_update
                        and si.on_wait
                        and all(
                            w.ant_name.startswith(pfx)
                            and w.wait_mode == "sem-ge-imm"
                            for w in si.on_wait
                        )
                    ):
                        removed += 1
                        continue
                keep.append(inst)
            blk.instructions[:] = keep
    return removed


def _build_program(n_steps, n_groups=2):
    import contextlib
    import concourse.tile as tile
    from concourse import bacc, mybir

    nc = bacc.Bacc(None)
    gsz = [NCOL // n_groups + (1 if i < NCOL % n_groups else 0) for i in range(n_groups)]
    ranges = []
    lo = 0
    for g in gsz:
        ranges.append((lo, lo + g))
        lo += g

    w_dram = nc.declare_dram_parameter("wmat", [128, 128], mybir.dt.bfloat16, False)
    x0_dram = nc.declare_dram_parameter("x0", [128, NCOL], mybir.dt.bfloat16, False)
    e_dram = nc.declare_dram_parameter(
        "econg", [128, n_steps * NCOL], mybir.dt.float8e5, False
    )
    snap1_dram = nc.declare_dram_parameter("snap1", [128, NCOL], mybir.dt.bfloat16, True)
    snap2_dram = nc.declare_dram_parameter("snap2", [128, NCOL], mybir.dt.bfloat16, True)
    out_dram = nc.declare_dram_parameter("xout", [128, NCOL], mybir.dt.bfloat16, True)

    # e-stream chunk schedule: small first chunks so the scan starts early
    sched = []
    left = n_steps
    for sz in (1, 1, 2, 4, 8):
        if left > 0:
            sched.append(min(sz, left))
            left -= sched[-1]
    while left > 0:
        sched.append(min(16, left))
        left -= sched[-1]
    chunk_of_step = []
    for ci, sz in enumerate(sched):
        chunk_of_step += [ci] * sz
    chunk_base = np.cumsum([0] + sched[:-1])

    with tile.TileContext(nc) as tc:
        with contextlib.ExitStack() as ctx:
            const_pool = ctx.enter_context(tc.tile_pool(name="const", bufs=1))
            epool = ctx.enter_context(tc.tile_pool(name="emis", bufs=1))
            xpool = ctx.enter_context(tc.tile_pool(name="x", bufs=8))
            psum_pool = ctx.enter_context(
                tc.tile_pool(name="ps", bufs=2, space="PSUM")
            )
            fin_pool = ctx.enter_context(tc.tile_pool(name="fin", bufs=1))

            # startup-critical DMAs first on the sync queue (x0 gates the
            # first matmul), e-chunks ride the gpsimd SWDGE queue
            x0_sb = const_pool.tile([128, NCOL], mybir.dt.bfloat16, tag="x0")
            nc.sync.dma_start(x0_sb[:], x0_dram[:])
            w_sb = const_pool.tile([128, 128], mybir.dt.bfloat16, tag="w")
            nc.sync.dma_start(w_sb[:], w_dram[:])

            e_tiles = []
            for ci, sz in enumerate(sched):
                et = epool.tile([128, sz * NCOL], mybir.dt.float8e5, tag=f"e{ci}")
                lo = int(chunk_base[ci]) * NCOL
                eng = nc.sync if ci % 2 == 0 else nc.gpsimd
                eng.dma_start(et[:], e_dram[:, lo : lo + sz * NCOL])
                e_tiles.append(et)

            dummy = fin_pool.tile([1, 4], mybir.dt.bfloat16, tag="dummy")

            xs = [(x0_sb, lo) for (lo, hi) in ranges]
            seen_chunk = -1
            for k in range(n_steps):
                step = k + 1  # local step index (1-based)
                ci = chunk_of_step[k]
                off = (k - int(chunk_base[ci])) * NCOL
                for g, (lo, hi) in enumerate(ranges):
                    cw = hi - lo
                    xt, xo = xs[g]
                    ps = psum_pool.tile([128, cw], mybir.dt.float32, tag=f"ps{g}")
                    for sub in range(0, cw, 512):
                        se = min(cw, sub + 512)
                        nc.tensor.matmul(
                            ps[:, sub:se],
                            w_sb[:],
                            xt[:, xo + sub : xo + se],
                            start=True,
                            stop=True,
                        )
                    xn = xpool.tile([128, cw], mybir.dt.bfloat16, tag=f"x{g}")
                    nc.vector.tensor_mul(
                        xn[:], ps[:], e_tiles[ci][:, off + lo : off + hi]
                    )
                    xs[g] = (xn, 0)
                if step == TAU:
                    for g, (lo, hi) in enumerate(ranges):
                        xt, xo = xs[g]
                        nc.sync.dma_start(
                            snap1_dram[:, lo:hi], xt[:, xo : xo + (hi - lo)]
                        )
                if step == TAU_F:
                    for g, (lo, hi) in enumerate(ranges):
                        xt, xo = xs[g]
                        nc.sync.dma_start(
                            snap2_dram[:, lo:hi], xt[:, xo : xo + (hi - lo)]
                        )

            for g, (lo, hi) in enumerate(ranges):
                xt, xo = xs[g]
                nc.sync.dma_start(out_dram[:, lo:hi], xt[:, xo : xo + (hi - lo)])

    nc.compile()
    if os.environ.get("HMM_STRIP_EV", "1") == "1":
        _strip_self_wait_events(nc)
    return nc


def _host_tables():
    """Chain->column mapping tables.

    Chain (seq b, j) for j in 0..NCH-1. Column layout: col = b*(NCH//2) + p
    holds chain 2p (partitions 0:64) and 2p+1 (partitions 64:128) of seq b
    when NCH even; handled generically via flat chain index."""
    # flat chain index c = b*NCH + j; column = c // 2, half = c % 2
    return None


def _host_prep(log_trans, log_emit, obvs):
    log_trans = np.asarray(log_trans, dtype=np.float64)
    log_emit = np.asarray(log_emit, dtype=np.float64)
    obvs = np.asarray(obvs).astype(np.int64)

    Ttil = np.exp(log_trans[1:, 1:])
    trans0 = np.exp(log_trans[0, 1:])
    E = np.exp(log_emit[1:, :] + C_SHIFT)  # [64,1024]
    E8 = E.astype(FP8)

    wmat = np.zeros((128, 128), dtype=np.float64)
    wmat[0:64, 0:64] = Ttil
    wmat[64:128, 64:128] = Ttil
    wmat = wmat.astype(BF16)

    # warm init: Perron vector of the mean-emission operator
    M = E.mean(axis=1)[:, None] * Ttil.T
    v = np.full(64, 1.0 / 64)
    for _ in range(100):
        v = M @ v
        v /= v.sum()
    warm = v

    starts = np.array(STARTS)  # [NCH]
    i_idx = np.arange(1, N_STEPS + 1)
    tok = starts[:, None] + i_idx[None, :]  # [NCH, n] global t per local step

    per_core = []
    for m in range(N_CORES):
        s0 = m * SEQ_PER_CORE
        obs_c = obvs[s0 : s0 + SEQ_PER_CORE, :]  # [16, T]

        # x0: [128, NCOL]; flat chain c = b*NCH + j -> column c//2, half c%2
        x0f = np.tile(warm[:, None], (1, SEQ_PER_CORE * NCH))
        x0f = x0f.reshape(64, SEQ_PER_CORE, NCH).copy()
        x0f[:, :, 0] = E[:, obs_c[:, 0]] * trans0[:, None]
        x0f = x0f.reshape(64, SEQ_PER_CORE * NCH)  # [64, nchains] b-major
        x0 = np.empty((128, NCOL), dtype=np.float64)
        x0[0:64, :] = x0f[:, 0::2]
        x0[64:128, :] = x0f[:, 1::2]
        per = {"wmat": wmat, "x0": x0.astype(BF16)}

        # e-stream: [128, n*NCOL] step-major fp8
        # chain c at local step i uses E8[:, obs[b, tok[j, i]]]
        em = E8[:, obs_c[:, tok]]  # [64, 16, NCH, n]
        em = em.reshape(64, SEQ_PER_CORE * NCH, N_STEPS)  # [64, nchains, n]
        econg = np.empty((128, N_STEPS, NCOL), dtype=FP8)
        econg[0:64] = np.transpose(em[:, 0::2, :], (0, 2, 1))
        econg[64:128] = np.transpose(em[:, 1::2, :], (0, 2, 1))
        per["econg"] = np.ascontiguousarray(econg.reshape(128, N_STEPS * NCOL))
        per_core.append(per)
    return per_core, warm


def _unpack_cols(arr):
    """[128, NCOL] -> [64, nchains] (b-major flat chain order)."""
    nch_flat = NCOL * 2
    out = np.empty((64, nch_flat), dtype=np.float64)
    out[:, 0::2] = arr[0:64, :]
    out[:, 1::2] = arr[64:128, :]
    return out


def _host_stitch(results, log_trans):
    log_trans = np.asarray(log_trans, dtype=np.float64)
    w_til = np.exp(log_trans[1:, 0] + 99.0)

    logZ = np.zeros(B, dtype=np.float64)
    for m in range(N_CORES):
        fin = _unpack_cols(np.asarray(results[m]["xout"], dtype=np.float64))
        sn1 = _unpack_cols(np.asarray(results[m]["snap1"], dtype=np.float64))
        sn2 = _unpack_cols(np.asarray(results[m]["snap2"], dtype=np.float64))
        fin = fin.reshape(64, SEQ_PER_CORE, NCH)
        sn1 = sn1.reshape(64, SEQ_PER_CORE, NCH)
        sn2 = sn2.reshape(64, SEQ_PER_CORE, NCH)
        for bb in range(SEQ_PER_CORE):
            logL = 0.0
            for j in range(1, NCH):
                snap = sn2[:, bb, j] if (j == NCH - 1 and TAU_F != TAU) else sn1[:, bb, j]
                fp = fin[:, bb, j - 1]
                rho = np.dot(snap, fp) / np.dot(fp, fp)
                logL -= np.log(rho)
            z = np.dot(w_til, fin[:, bb, NCH - 1])
            logZ[m * SEQ_PER_CORE + bb] = logL + np.log(z) - T * C_SHIFT - 99.0
    return logZ


def _run(nc, per_core, trace=False):
    from concourse.bass_utils import run_bass_kernel_spmd

    return run_bass_kernel_spmd(
        nc, per_core, list(range(N_CORES)), trace=trace, trace_cores=[0]
    )


def kernel(log_trans, log_emit, log_pi, obvs):
    nc = _build_program(N_STEPS)
    per_core, _ = _host_prep(log_trans, log_emit, obvs)
    res = _run(nc, per_core)
    out = _host_stitch(res.results, log_trans)
    return out.astype(np.float32)


# revision 22
# speedup vs baseline: 1.0546x; 1.0150x over previous
"""Trainium2 Bass kernel v2: batched HMM log-forward via overlapped
forward-chain scan.

Linear-space chain over 64 states (state 0 bookend dropped):
  a_1 = E[:,obs_0]*trans0,  a_{t} = D_{E[:,obs_t]} Ttil^T a_{t-1},
  Z*e^{99+8192*C} = w~ . a_{8191}   (E drift-compensated by C_SHIFT).

Scheme: chains of length N_STEPS cover windows (a_j, a_j+N_STEPS],
a_j = j*STRIDE, plus a final chain ending exactly at 8191. Chain 0 starts
from the exact init; later chains from a warm init vector. After TAU
warmup steps a chain's state is parallel to the true alpha direction
(Perron-Frobenius contraction), so the scalar
  rho_j = <snap_j(tau_j), fin_{j-1}> / <fin_{j-1}, fin_{j-1}>
links consecutive chains and logZ telescopes: logZ = -sum_j log rho_j
+ log(w~ . fin_last) - 8192*C_SHIFT - 99.

Work vs fwd+bwd pairing drops ~1.45x (redundancy n/(n-tau) vs 2.0).

Device: 2 fwd chains per 128-partition column (block-diag weight
diag(Ttil,Ttil)), scan N_STEPS iterations over NCOL columns in G=2
groups; per step per group one PE matmul -> PSUM and one DVE multiply
PSUM*e(fp8) -> SBUF bf16. Snapshots at local steps TAU and TAU_F plus
the final state are DMA'd out; the stitch runs on host in float64.
"""

import os
import numpy as np
import ml_dtypes

B, T, S, V = 128, 8192, 65, 1024
N_CORES = 8
SEQ_PER_CORE = B // N_CORES  # 16
C_SHIFT = 6.9418
BF16 = ml_dtypes.bfloat16
FP8 = ml_dtypes.float8_e5m2

N_STEPS = int(os.environ.get("HMM_N", "65"))
TAU = int(os.environ.get("HMM_TAU", "1"))
STRIDE = N_STEPS - TAU
TM = T - 1  # matrices t = 1..8191


def _chain_starts():
    k_int = (TM - N_STEPS) // STRIDE
    starts = [j * STRIDE for j in range(k_int + 1)]
    if starts[-1] + N_STEPS < TM:
        starts.append(TM - N_STEPS)
    return starts


STARTS = _chain_starts()
NCH = len(STARTS)  # chains per sequence
TAU_F = STARTS[-2] + N_STEPS - STARTS[-1] if STARTS[-1] != STARTS[-2] + STRIDE else TAU
NCHAIN_CORE = NCH * SEQ_PER_CORE
assert NCHAIN_CORE % 2 == 0
NCOL = NCHAIN_CORE // 2  # 2 chains per column


def _strip_self_wait_events(nc):
    """Remove InstEventSemaphore instrs that only wait on the issuing
    engine's own semaphore (trivially-true WAW guards)."""
    eng_prefix = {
        "EngineType.DVE": "DVE_",
        "EngineType.PE": "PE_",
        "EngineType.Activation": "Activation_",
        "EngineType.Pool": "Pool_",
    }
    removed = 0
    for fn in nc.m.functions:
        for blk in fn.blocks:
            keep = []
            for inst in blk.instructions:
                if type(inst).__name__ == "InstEventSemaphore":
                    pfx = eng_prefix.get(str(getattr(inst, "engine", "")), None)
                    si = inst.sync_info
                    if (
                        pfx is not None
                        and si
                        and not si.on_update
                        and si.on_wait
                        and all(
                            w.ant_name.startswith(pfx)
                            and w.wait_mode == "sem-ge-imm"
                            for w in si.on_wait
                        )
                    ):
                        removed += 1
                        continue
                keep.append(inst)
            blk.instructions[:] = keep
    return removed


def _build_program(n_steps, n_groups=2):
    import contextlib
    import concourse.tile as tile
    from concourse import bacc, mybir

    nc = bacc.Bacc(None)
    gsz = [NCOL // n_groups + (1 if i < NCOL % n_groups else 0) for i in range(n_groups)]
    ranges = []
    lo = 0
    for g in gsz:
        ranges.append((lo, lo + g))
        lo += g

    w_dram = nc.declare_dram_parameter("wmat", [128, 128], mybir.dt.bfloat16, False)
    x0_dram = nc.declare_dram_parameter("x0", [128, NCOL], mybir.dt.bfloat16, False)
    e_dram = nc.declare_dram_parameter(
        "econg", [128, n_steps * NCOL], mybir.dt.float8e5, False
    )
    snap1_dram = nc.declare_dram_parameter("snap1", [128, NCOL], mybir.dt.bfloat16, True)
    snap2_dram = nc.declare_dram_parameter("snap2", [128, NCOL], mybir.dt.bfloat16, True)
    out_dram = nc.declare_dram_parameter("xout", [128, NCOL], mybir.dt.bfloat16, True)

    # e-stream chunk schedule: small first chunks so the scan starts early
    sched = []
    left = n_steps
    for sz in (1, 1, 2, 4, 8):
        if left > 0:
            sched.append(min(sz, left))
            left -= sched[-1]
    while left > 0:
        sched.append(min(16, left))
        left -= sched[-1]
    chunk_of_step = []
    for ci, sz in enumerate(sched):
        chunk_of_step += [ci] * sz
    chunk_base = np.cumsum([0] + sched[:-1])

    with tile.TileContext(nc) as tc:
        with contextlib.ExitStack() as ctx:
            const_pool = ctx.enter_context(tc.tile_pool(name="const", bufs=1))
            epool = ctx.enter_context(tc.tile_pool(name="emis", bufs=1))
            xpool = ctx.enter_context(tc.tile_pool(name="x", bufs=8))
            psum_pool = ctx.enter_context(
                tc.tile_pool(name="ps", bufs=4, space="PSUM")
            )
            fin_pool = ctx.enter_context(tc.tile_pool(name="fin", bufs=1))

            # startup-critical DMAs first on the sync queue (x0 gates the
            # first matmul), e-chunks ride the gpsimd SWDGE queue
            x0_sb = const_pool.tile([128, NCOL], mybir.dt.bfloat16, tag="x0")
            nc.sync.dma_start(x0_sb[:], x0_dram[:])
            w_sb = const_pool.tile([128, 128], mybir.dt.bfloat16, tag="w")
            nc.sync.dma_start(w_sb[:], w_dram[:])

            e_tiles = []
            for ci, sz in enumerate(sched):
                et = epool.tile([128, sz * NCOL], mybir.dt.float8e5, tag=f"e{ci}")
                lo = int(chunk_base[ci]) * NCOL
                eng = nc.sync if ci % 2 == 0 else nc.gpsimd
                eng.dma_start(et[:], e_dram[:, lo : lo + sz * NCOL])
                e_tiles.append(et)

            dummy = fin_pool.tile([1, 4], mybir.dt.bfloat16, tag="dummy")

            xs = [(x0_sb, lo) for (lo, hi) in ranges]
            seen_chunk = -1
            for k in range(n_steps):
                step = k + 1  # local step index (1-based)
                ci = chunk_of_step[k]
                off = (k - int(chunk_base[ci])) * NCOL
                for g, (lo, hi) in enumerate(ranges):
                    cw = hi - lo
                    xt, xo = xs[g]
                    ps = psum_pool.tile([128, cw], mybir.dt.float32, tag=f"ps{g}")
                    for sub in range(0, cw, 512):
                        se = min(cw, sub + 512)
                        nc.tensor.matmul(
                            ps[:, sub:se],
                            w_sb[:],
                            xt[:, xo + sub : xo + se],
                            start=True,
                            stop=True,
                        )
                    xn = xpool.tile([128, cw], mybir.dt.bfloat16, tag=f"x{g}")
                    nc.vector.tensor_mul(
                        xn[:], ps[:], e_tiles[ci][:, off + lo : off + hi]
                    )
                    xs[g] = (xn, 0)
                if step == TAU:
                    for g, (lo, hi) in enumerate(ranges):
                        xt, xo = xs[g]
                        nc.sync.dma_start(
                            snap1_dram[:, lo:hi], xt[:, xo : xo + (hi - lo)]
                        )
                if step == TAU_F:
                    for g, (lo, hi) in enumerate(ranges):
                        xt, xo = xs[g]
                        nc.sync.dma_start(
                            snap2_dram[:, lo:hi], xt[:, xo : xo + (hi - lo)]
                        )

            for g, (lo, hi) in enumerate(ranges):
                xt, xo = xs[g]
                nc.sync.dma_start(out_dram[:, lo:hi], xt[:, xo : xo + (hi - lo)])

    nc.compile()
    if os.environ.get("HMM_STRIP_EV", "1") == "1":
        _strip_self_wait_events(nc)
    return nc


def _host_tables():
    """Chain->column mapping tables.

    Chain (seq b, j) for j in 0..NCH-1. Column layout: col = b*(NCH//2) + p
    holds chain 2p (partitions 0:64) and 2p+1 (partitions 64:128) of seq b
    when NCH even; handled generically via flat chain index."""
    # flat chain index c = b*NCH + j; column = c // 2, half = c % 2
    return None


def _host_prep(log_trans, log_emit, obvs):
    log_trans = np.asarray(log_trans, dtype=np.float64)
    log_emit = np.asarray(log_emit, dtype=np.float64)
    obvs = np.asarray(obvs).astype(np.int64)

    Ttil = np.exp(log_trans[1:, 1:])
    trans0 = np.exp(log_trans[0, 1:])
    E = np.exp(log_emit[1:, :] + C_SHIFT)  # [64,1024]
    E8 = E.astype(FP8)

    wmat = np.zeros((128, 128), dtype=np.float64)
    wmat[0:64, 0:64] = Ttil
    wmat[64:128, 64:128] = Ttil
    wmat = wmat.astype(BF16)

    # warm init: Perron vector of the mean-emission operator
    M = E.mean(axis=1)[:, None] * Ttil.T
    v = np.full(64, 1.0 / 64)
    for _ in range(100):
        v = M @ v
        v /= v.sum()
    warm = v

    starts = np.array(STARTS)  # [NCH]
    i_idx = np.arange(1, N_STEPS + 1)
    tok = starts[:, None] + i_idx[None, :]  # [NCH, n] global t per local step

    per_core = []
    for m in range(N_CORES):
        s0 = m * SEQ_PER_CORE
        obs_c = obvs[s0 : s0 + SEQ_PER_CORE, :]  # [16, T]

        # x0: [128, NCOL]; flat chain c = b*NCH + j -> column c//2, half c%2
        x0f = np.tile(warm[:, None], (1, SEQ_PER_CORE * NCH))
        x0f = x0f.reshape(64, SEQ_PER_CORE, NCH).copy()
        x0f[:, :, 0] = E[:, obs_c[:, 0]] * trans0[:, None]
        x0f = x0f.reshape(64, SEQ_PER_CORE * NCH)  # [64, nchains] b-major
        x0 = np.empty((128, NCOL), dtype=np.float64)
        x0[0:64, :] = x0f[:, 0::2]
        x0[64:128, :] = x0f[:, 1::2]
        per = {"wmat": wmat, "x0": x0.astype(BF16)}

        # e-stream: [128, n*NCOL] step-major fp8
        # chain c at local step i uses E8[:, obs[b, tok[j, i]]]
        em = E8[:, obs_c[:, tok]]  # [64, 16, NCH, n]
        em = em.reshape(64, SEQ_PER_CORE * NCH, N_STEPS)  # [64, nchains, n]
        econg = np.empty((128, N_STEPS, NCOL), dtype=FP8)
        econg[0:64] = np.transpose(em[:, 0::2, :], (0, 2, 1))
        econg[64:128] = np.transpose(em[:, 1::2, :], (0, 2, 1))
        per["econg"] = np.ascontiguousarray(econg.reshape(128, N_STEPS * NCOL))
        per_core.append(per)
    return per_core, warm


def _unpack_cols(arr):
    """[128, NCOL] -> [64, nchains] (b-major flat chain order)."""
    nch_flat = NCOL * 2
    out = np.empty((64, nch_flat), dtype=np.float64)
    out[:, 0::2] = arr[0:64, :]
    out[:, 1::2] = arr[64:128, :]
    return out


def _host_stitch(results, log_trans):
    log_trans = np.asarray(log_trans, dtype=np.float64)
    w_til = np.exp(log_trans[1:, 0] + 99.0)

    logZ = np.zeros(B, dtype=np.float64)
    for m in range(N_CORES):
        fin = _unpack_cols(np.asarray(results[m]["xout"], dtype=np.float64))
        sn1 = _unpack_cols(np.asarray(results[m]["snap1"], dtype=np.float64))
        sn2 = _unpack_cols(np.asarray(results[m]["snap2"], dtype=np.float64))
        fin = fin.reshape(64, SEQ_PER_CORE, NCH)
        sn1 = sn1.reshape(64, SEQ_PER_CORE, NCH)
        sn2 = sn2.reshape(64, SEQ_PER_CORE, NCH)
        for bb in range(SEQ_PER_CORE):
            logL = 0.0
            for j in range(1, NCH):
                snap = sn2[:, bb, j] if (j == NCH - 1 and TAU_F != TAU) else sn1[:, bb, j]
                fp = fin[:, bb, j - 1]
                rho = np.dot(snap, fp) / np.dot(fp, fp)
                logL -= np.log(rho)
            z = np.dot(w_til, fin[:, bb, NCH - 1])
            logZ[m * SEQ_PER_CORE + bb] = logL + np.log(z) - T * C_SHIFT - 99.0
    return logZ


def _run(nc, per_core, trace=False):
    from concourse.bass_utils import run_bass_kernel_spmd

    return run_bass_kernel_spmd(
        nc, per_core, list(range(N_CORES)), trace=trace, trace_cores=[0]
    )


def kernel(log_trans, log_emit, log_pi, obvs):
    nc = _build_program(N_STEPS)
    per_core, _ = _host_prep(log_trans, log_emit, obvs)
    res = _run(nc, per_core)
    out = _host_stitch(res.results, log_trans)
    return out.astype(np.float32)


# revision 23
# speedup vs baseline: 1.0743x; 1.0186x over previous
"""Trainium2 Bass kernel v2: batched HMM log-forward via overlapped
forward-chain scan.

Linear-space chain over 64 states (state 0 bookend dropped):
  a_1 = E[:,obs_0]*trans0,  a_{t} = D_{E[:,obs_t]} Ttil^T a_{t-1},
  Z*e^{99+8192*C} = w~ . a_{8191}   (E drift-compensated by C_SHIFT).

Scheme: chains of length N_STEPS cover windows (a_j, a_j+N_STEPS],
a_j = j*STRIDE, plus a final chain ending exactly at 8191. Chain 0 starts
from the exact init; later chains from a warm init vector. After TAU
warmup steps a chain's state is parallel to the true alpha direction
(Perron-Frobenius contraction), so the scalar
  rho_j = <snap_j(tau_j), fin_{j-1}> / <fin_{j-1}, fin_{j-1}>
links consecutive chains and logZ telescopes: logZ = -sum_j log rho_j
+ log(w~ . fin_last) - 8192*C_SHIFT - 99.

Work vs fwd+bwd pairing drops ~1.45x (redundancy n/(n-tau) vs 2.0).

Device: 2 fwd chains per 128-partition column (block-diag weight
diag(Ttil,Ttil)), scan N_STEPS iterations over NCOL columns in G=2
groups; per step per group one PE matmul -> PSUM and one DVE multiply
PSUM*e(fp8) -> SBUF bf16. Snapshots at local steps TAU and TAU_F plus
the final state are DMA'd out; the stitch runs on host in float64.
"""

import os
import numpy as np
import ml_dtypes

B, T, S, V = 128, 8192, 65, 1024
N_CORES = 8
SEQ_PER_CORE = B // N_CORES  # 16
C_SHIFT = 6.9418
BF16 = ml_dtypes.bfloat16
FP8 = ml_dtypes.float8_e5m2

N_STEPS = int(os.environ.get("HMM_N", "65"))
TAU = int(os.environ.get("HMM_TAU", "1"))
STRIDE = N_STEPS - TAU
TM = T - 1  # matrices t = 1..8191


def _chain_starts():
    k_int = (TM - N_STEPS) // STRIDE
    starts = [j * STRIDE for j in range(k_int + 1)]
    if starts[-1] + N_STEPS < TM:
        starts.append(TM - N_STEPS)
    return starts


STARTS = _chain_starts()
NCH = len(STARTS)  # chains per sequence
TAU_F = STARTS[-2] + N_STEPS - STARTS[-1] if STARTS[-1] != STARTS[-2] + STRIDE else TAU
NCHAIN_CORE = NCH * SEQ_PER_CORE
assert NCHAIN_CORE % 2 == 0
NCOL = NCHAIN_CORE // 2  # 2 chains per column


def _strip_self_wait_events(nc):
    """Remove InstEventSemaphore instrs that only wait on the issuing
    engine's own semaphore (trivially-true WAW guards)."""
    eng_prefix = {
        "EngineType.DVE": "DVE_",
        "EngineType.PE": "PE_",
        "EngineType.Activation": "Activation_",
        "EngineType.Pool": "Pool_",
    }
    removed = 0
    for fn in nc.m.functions:
        for blk in fn.blocks:
            keep = []
            for inst in blk.instructions:
                if type(inst).__name__ == "InstEventSemaphore":
                    pfx = eng_prefix.get(str(getattr(inst, "engine", "")), None)
                    si = inst.sync_info
                    if (
                        pfx is not None
                        and si
                        and not si.on_update
                        and si.on_wait
                        and all(
                            w.ant_name.startswith(pfx)
                            and w.wait_mode == "sem-ge-imm"
                            for w in si.on_wait
                        )
                    ):
                        removed += 1
                        continue
                keep.append(inst)
            blk.instructions[:] = keep
    return removed


def _build_program(n_steps, n_groups=2):
    import contextlib
    import concourse.tile as tile
    from concourse import bacc, mybir

    nc = bacc.Bacc(None)
    gsz = [NCOL // n_groups + (1 if i < NCOL % n_groups else 0) for i in range(n_groups)]
    ranges = []
    lo = 0
    for g in gsz:
        ranges.append((lo, lo + g))
        lo += g

    w_dram = nc.declare_dram_parameter("wmat", [128, 128], mybir.dt.bfloat16, False)
    x0_dram = nc.declare_dram_parameter("x0", [128, NCOL], mybir.dt.bfloat16, False)
    e_dram = nc.declare_dram_parameter(
        "econg", [128, n_steps * NCOL], mybir.dt.float8e5, False
    )
    snap1_dram = nc.declare_dram_parameter("snap1", [128, NCOL], mybir.dt.bfloat16, True)
    snap2_dram = nc.declare_dram_parameter("snap2", [128, NCOL], mybir.dt.bfloat16, True)
    out_dram = nc.declare_dram_parameter("xout", [128, NCOL], mybir.dt.bfloat16, True)

    # e-stream chunk schedule: small first chunks so the scan starts early
    sched = []
    left = n_steps
    for sz in (1, 1, 2, 4, 8):
        if left > 0:
            sched.append(min(sz, left))
            left -= sched[-1]
    while left > 0:
        sched.append(min(16, left))
        left -= sched[-1]
    chunk_of_step = []
    for ci, sz in enumerate(sched):
        chunk_of_step += [ci] * sz
    chunk_base = np.cumsum([0] + sched[:-1])

    with tile.TileContext(nc) as tc:
        with contextlib.ExitStack() as ctx:
            const_pool = ctx.enter_context(tc.tile_pool(name="const", bufs=1))
            epool = ctx.enter_context(tc.tile_pool(name="emis", bufs=1))
            xpool = ctx.enter_context(tc.tile_pool(name="x", bufs=8))
            psum_pool = ctx.enter_context(
                tc.tile_pool(name="ps", bufs=4, space="PSUM")
            )
            fin_pool = ctx.enter_context(tc.tile_pool(name="fin", bufs=1))

            # startup-critical DMAs first on the sync queue (x0 gates the
            # first matmul), e-chunks ride the gpsimd SWDGE queue
            x0_sb = const_pool.tile([128, NCOL], mybir.dt.bfloat16, tag="x0")
            nc.sync.dma_start(x0_sb[:], x0_dram[:])
            w_sb = const_pool.tile([128, 128], mybir.dt.bfloat16, tag="w")
            nc.sync.dma_start(w_sb[:], w_dram[:])

            e_tiles = []
            for ci, sz in enumerate(sched):
                et = epool.tile([128, sz * NCOL], mybir.dt.float8e5, tag=f"e{ci}")
                lo = int(chunk_base[ci]) * NCOL
                eng = nc.gpsimd if ci % 2 == 0 else nc.sync
                eng.dma_start(et[:], e_dram[:, lo : lo + sz * NCOL])
                e_tiles.append(et)

            dummy = fin_pool.tile([1, 4], mybir.dt.bfloat16, tag="dummy")

            xs = [(x0_sb, lo) for (lo, hi) in ranges]
            seen_chunk = -1
            for k in range(n_steps):
                step = k + 1  # local step index (1-based)
                ci = chunk_of_step[k]
                off = (k - int(chunk_base[ci])) * NCOL
                for g, (lo, hi) in enumerate(ranges):
                    cw = hi - lo
                    xt, xo = xs[g]
                    ps = psum_pool.tile([128, cw], mybir.dt.float32, tag=f"ps{g}")
                    for sub in range(0, cw, 512):
                        se = min(cw, sub + 512)
                        nc.tensor.matmul(
                            ps[:, sub:se],
                            w_sb[:],
                            xt[:, xo + sub : xo + se],
                            start=True,
                            stop=True,
                        )
                    xn = xpool.tile([128, cw], mybir.dt.bfloat16, tag=f"x{g}")
                    nc.vector.tensor_mul(
                        xn[:], ps[:], e_tiles[ci][:, off + lo : off + hi]
                    )
                    xs[g] = (xn, 0)
                if step == TAU:
                    for g, (lo, hi) in enumerate(ranges):
                        xt, xo = xs[g]
                        nc.sync.dma_start(
                            snap1_dram[:, lo:hi], xt[:, xo : xo + (hi - lo)]
                        )
                if step == TAU_F:
                    for g, (lo, hi) in enumerate(ranges):
                        xt, xo = xs[g]
                        nc.sync.dma_start(
                            snap2_dram[:, lo:hi], xt[:, xo : xo + (hi - lo)]
                        )

            for g, (lo, hi) in enumerate(ranges):
                xt, xo = xs[g]
                nc.sync.dma_start(out_dram[:, lo:hi], xt[:, xo : xo + (hi - lo)])

    nc.compile()
    if os.environ.get("HMM_STRIP_EV", "1") == "1":
        _strip_self_wait_events(nc)
    return nc


def _host_tables():
    """Chain->column mapping tables.

    Chain (seq b, j) for j in 0..NCH-1. Column layout: col = b*(NCH//2) + p
    holds chain 2p (partitions 0:64) and 2p+1 (partitions 64:128) of seq b
    when NCH even; handled generically via flat chain index."""
    # flat chain index c = b*NCH + j; column = c // 2, half = c % 2
    return None


def _host_prep(log_trans, log_emit, obvs):
    log_trans = np.asarray(log_trans, dtype=np.float64)
    log_emit = np.asarray(log_emit, dtype=np.float64)
    obvs = np.asarray(obvs).astype(np.int64)

    Ttil = np.exp(log_trans[1:, 1:])
    trans0 = np.exp(log_trans[0, 1:])
    E = np.exp(log_emit[1:, :] + C_SHIFT)  # [64,1024]
    E8 = E.astype(FP8)

    wmat = np.zeros((128, 128), dtype=np.float64)
    wmat[0:64, 0:64] = Ttil
    wmat[64:128, 64:128] = Ttil
    wmat = wmat.astype(BF16)

    # warm init: Perron vector of the mean-emission operator
    M = E.mean(axis=1)[:, None] * Ttil.T
    v = np.full(64, 1.0 / 64)
    for _ in range(100):
        v = M @ v
        v /= v.sum()
    warm = v

    starts = np.array(STARTS)  # [NCH]
    i_idx = np.arange(1, N_STEPS + 1)
    tok = starts[:, None] + i_idx[None, :]  # [NCH, n] global t per local step

    per_core = []
    for m in range(N_CORES):
        s0 = m * SEQ_PER_CORE
        obs_c = obvs[s0 : s0 + SEQ_PER_CORE, :]  # [16, T]

        # x0: [128, NCOL]; flat chain c = b*NCH + j -> column c//2, half c%2
        x0f = np.tile(warm[:, None], (1, SEQ_PER_CORE * NCH))
        x0f = x0f.reshape(64, SEQ_PER_CORE, NCH).copy()
        x0f[:, :, 0] = E[:, obs_c[:, 0]] * trans0[:, None]
        x0f = x0f.reshape(64, SEQ_PER_CORE * NCH)  # [64, nchains] b-major
        x0 = np.empty((128, NCOL), dtype=np.float64)
        x0[0:64, :] = x0f[:, 0::2]
        x0[64:128, :] = x0f[:, 1::2]
        per = {"wmat": wmat, "x0": x0.astype(BF16)}

        # e-stream: [128, n*NCOL] step-major fp8
        # chain c at local step i uses E8[:, obs[b, tok[j, i]]]
        em = E8[:, obs_c[:, tok]]  # [64, 16, NCH, n]
        em = em.reshape(64, SEQ_PER_CORE * NCH, N_STEPS)  # [64, nchains, n]
        econg = np.empty((128, N_STEPS, NCOL), dtype=FP8)
        econg[0:64] = np.transpose(em[:, 0::2, :], (0, 2, 1))
        econg[64:128] = np.transpose(em[:, 1::2, :], (0, 2, 1))
        per["econg"] = np.ascontiguousarray(econg.reshape(128, N_STEPS * NCOL))
        per_core.append(per)
    return per_core, warm


def _unpack_cols(arr):
    """[128, NCOL] -> [64, nchains] (b-major flat chain order)."""
    nch_flat = NCOL * 2
    out = np.empty((64, nch_flat), dtype=np.float64)
    out[:, 0::2] = arr[0:64, :]
    out[:, 1::2] = arr[64:128, :]
    return out


def _host_stitch(results, log_trans):
    log_trans = np.asarray(log_trans, dtype=np.float64)
    w_til = np.exp(log_trans[1:, 0] + 99.0)

    logZ = np.zeros(B, dtype=np.float64)
    for m in range(N_CORES):
        fin = _unpack_cols(np.asarray(results[m]["xout"], dtype=np.float64))
        sn1 = _unpack_cols(np.asarray(results[m]["snap1"], dtype=np.float64))
        sn2 = _unpack_cols(np.asarray(results[m]["snap2"], dtype=np.float64))
        fin = fin.reshape(64, SEQ_PER_CORE, NCH)
        sn1 = sn1.reshape(64, SEQ_PER_CORE, NCH)
        sn2 = sn2.reshape(64, SEQ_PER_CORE, NCH)
        for bb in range(SEQ_PER_CORE):
            logL = 0.0
            for j in range(1, NCH):
                snap = sn2[:, bb, j] if (j == NCH - 1 and TAU_F != TAU) else sn1[:, bb, j]
                fp = fin[:, bb, j - 1]
                rho = np.dot(snap, fp) / np.dot(fp, fp)
                logL -= np.log(rho)
            z = np.dot(w_til, fin[:, bb, NCH - 1])
            logZ[m * SEQ_PER_CORE + bb] = logL + np.log(z) - T * C_SHIFT - 99.0
    return logZ


def _run(nc, per_core, trace=False):
    from concourse.bass_utils import run_bass_kernel_spmd

    return run_bass_kernel_spmd(
        nc, per_core, list(range(N_CORES)), trace=trace, trace_cores=[0]
    )


def kernel(log_trans, log_emit, log_pi, obvs):
    nc = _build_program(N_STEPS)
    per_core, _ = _host_prep(log_trans, log_emit, obvs)
    res = _run(nc, per_core)
    out = _host_stitch(res.results, log_trans)
    return out.astype(np.float32)
